# revision 6
# baseline (speedup 1.0000x reference)
"""Trainium2 8-core MoE layer kernel (token x ffn sharded dense FFN, Bass/Tile).

Contract: kernel(**inputs) takes the full unsharded numpy inputs of the
MoE reference (hidden_states, router_w, w1, b1, w2, b2) and returns the
full [2, 1024, 2048] float32 output.

Key identity: the reference's experts all share one FFN (w1/b1/w2/b2 are
not per-expert), so for every kept (token, k) slot the expert output is
FFN(x[t]) and the combine collapses to
    out[t] = (sum_k kept_k(t) * gate_k(t)) * FFN(x[t]).
Routing therefore only determines a per-token scalar; the FFN itself is
a dense [N, H] pass.

Sharding: cores form pairs (2p, 2p+1) owning tokens [512p, 512p+512).
Within a pair the ffn dim F is split in half (4096 per core), so each
core streams only 32 MB of weights (the kernel is HBM-bound at ~220 GB/s
per core, while the PE floor is ~134 us per GEMM phase). Each core
computes partial y = gelu(x W1h^T + b1h) W2h^T for all 512 pair tokens;
a per-H-chunk bf16 ReduceScatter(add) over the pair then hands every
core the finished rows of its own 256 tokens (= global tokens
[256c, 256c+256), matching the host-side concat).

The fp32 router runs on each core's own 256 tokens; an AllGather shares
the [N, 4] routing decisions; a replicated capacity scan produces the
per-token weight w(t), indirect-gathered per core. That whole chain
rides on vector/gpsimd during F1; its two tiny matmul groups issue
between F1 and F2 on the tensor queue.
"""
import numpy as np
import ml_dtypes

import concourse.bass as bass
import concourse.mybir as mybir
import concourse.tile as tile

_PATCH_DOC = """Patch TileContext._drain_and_barrier: the stock version stuffs every
outstanding semaphore wait onto one SP Drain instruction; the installed
walrus rejects >1 sync wait per non-EventSemaphore instruction
("Too many sync wait commands"). Split the waits across a chain of SP
nops, then drain/barrier as before."""
import concourse.tile as tile_mod
from concourse.vector_clock import ScopedClock


def _patched_drain_and_barrier(self, tick_clock, wait_clock):
    nc = self.nc
    carrier = nc.sync.nop(nofuse=True, hint="drain_wait_carrier")
    wait_clock.add_sem_waits(
        carrier.ins, ScopedClock({None: tick_clock.global_clock})
    )
    waits = list(carrier.ins.sync_info.on_wait)
    if len(waits) > 1:
        carrier.ins.sync_info.on_wait = waits[:1]
        import bass_rust as _br
        for w in waits[1:]:
            extra = nc.sync.nop(nofuse=True, hint="drain_wait_carrier")
            extra.ins.sync_info = _br.SyncInfo(on_wait=[w], on_update=[])

    nc.sync.drain()
    nc.all_engine_barrier()
    assert self.sems is not None
    popped = nc._tile_sem_poison_stack.pop()
    assert popped is self._sem_poison
    nc.clear_and_free_semaphores(list(self.sems.allocated().values()))
    nc.all_engine_barrier()


def apply():
    tile_mod.TileContext._drain_and_barrier = _patched_drain_and_barrier


import concourse.mybir as mybir
import bass_rust as _br


def split_multi_waits(nc):
    """Walrus in this container accepts at most ONE sync wait per
    instruction. Hoist extra waits onto same-engine NoOps inserted
    immediately before the offending instruction."""
    ctr = 0
    for f in nc.m.functions:
        for b in f.blocks:
            insts = b.instructions
            need = any(
                inst.sync_info is not None and len(inst.sync_info.on_wait) > 1
                for inst in insts
            )
            if not need:
                continue
            out = []
            for inst in insts:
                si = inst.sync_info
                if si is not None and len(si.on_wait) > 1:
                    waits = list(si.on_wait)
                    for w in waits[:-1]:
                        nop = mybir.InstNoOp(name=f"I-wsplit-{ctr}", ins=[], outs=[])
                        ctr += 1
                        nop.engine = inst.engine
                        nop.sync_info = _br.SyncInfo(on_wait=[w], on_update=[])
                        out.append(nop)
                    si.on_wait = waits[-1:]
                out.append(inst)
            b.instructions = out
    return ctr


E, TOPK, CAP, H, F, N, NCORES = 8, 2, 512, 2048, 8192, 2048, 8
HT = H // 128                 # 16 hidden tiles
FH = F // 2                   # 4096 ffn columns per core
FTH = FH // 128               # 32 local ffn tiles
TOKC = N // NCORES            # 256 output tokens per core
TOKP = 2 * TOKC               # 512 tokens per pair
BI = N // 128                 # 16
NSEG = 4
SEGL = 2 * N // NSEG
# F2 H-chunks; the small last chunk keeps the exposed final
# ReduceScatter + epilogue short
CHUNKS = [(0, 512), (512, 512), (1024, 512), (1536, 384), (1920, 128)]

f32 = mybir.dt.float32
f16 = mybir.dt.float16
bf16 = mybir.dt.bfloat16
i32 = mybir.dt.int32
AOP = mybir.AluOpType
AFT = mybir.ActivationFunctionType
AX = mybir.AxisListType


def build_moe(nc: bass.Bass):
    xtm = nc.dram_tensor("xtm", [2, 128, H], f32, kind="ExternalInput")
    xtt = nc.dram_tensor("xtt", [128, HT * TOKP], bf16, kind="ExternalInput")
    rwT = nc.dram_tensor("rwT", [128, HT * E], f32, kind="ExternalInput")
    w1T = nc.dram_tensor("w1tt", [FTH, 128, HT * 128], bf16, kind="ExternalInput")
    w2q = nc.dram_tensor("w2q", [FTH, 128, H], bf16, kind="ExternalInput")
    b1t = nc.dram_tensor("b1t", [128, FTH], f32, kind="ExternalInput")
    b2r = nc.dram_tensor("b2r", [1, H], f32, kind="ExternalInput")
    cid = nc.dram_tensor("cid", [1, 1], f32, kind="ExternalInput")
    out = nc.dram_tensor("out", [TOKC, H], f32, kind="ExternalOutput")

    rloc = nc.dram_tensor("rloc", [TOKC, 4], f32)
    rall = nc.dram_tensor("rall", [N, 4], f32, addr_space="Shared")
    ebuf8 = nc.dram_tensor("ebuf8", [E, 2 * N], f32)
    e32d = nc.dram_tensor("e32d", [1, 32], f32)
    posd = nc.dram_tensor("posd", [1, 2 * N], f32)
    wd = nc.dram_tensor("wd", [N, 1], f32)
    pq = [nc.dram_tensor(f"pq{ci}", [TOKP, w], bf16)
          for ci, (o, w) in enumerate(CHUNKS)]
    arq = [nc.dram_tensor(f"arq{ci}", [TOKC, w], bf16)
           for ci, (o, w) in enumerate(CHUNKS)]

    with tile.TileContext(nc, num_cores=NCORES) as tc:
        with tc.tile_pool(name="persist", bufs=1) as persist:
            _body(nc, tc, persist, xtm, xtt, rwT, w1T, w2q, b1t, b2r,
                  cid, out, rloc, rall, ebuf8, e32d, posd, wd, pq, arq)
    return nc


def _body(nc, tc, persist, xtm, xtt, rwT, w1T, w2q, b1t, b2r, cid, out,
          rloc, rall, ebuf8, e32d, posd, wd, pq, arq):
    RG = [list(range(NCORES))]
    PRG = [[2 * p, 2 * p + 1] for p in range(NCORES // 2)]
    sc = persist

    # ---- persistent tiles ----
    b2b = persist.tile([128, H], f32, tag="b2b")
    cidb = persist.tile([128, 1], f32, tag="cidb")
    b1sb = persist.tile([128, FTH], f32, tag="b1sb")
    rws = persist.tile([128, HT * E], f32, tag="rws")
    xcT = persist.tile([128, HT * TOKP], bf16, tag="xcT")
    iotap = persist.tile([128, 1], f32, tag="iotap")
    wt = persist.tile([128, 2], f32, tag="wt")
    widx = persist.tile([128, 2], i32, tag="widx")

    # x loads split across the two fast queues so the router and F1 can
    # start immediately; everything else on gpsimd
    QX = HT * TOKP // 2
    nc.sync.dma_start(out=xcT[:, 0:QX], in_=xtt[:, 0:QX])
    nc.scalar.dma_start(out=xcT[:, QX:2 * QX], in_=xtt[:, QX:2 * QX])
    nc.gpsimd.dma_start(out=rws[:], in_=rwT[:, :])
    nc.gpsimd.dma_start(out=cidb[:], in_=cid[0:1, :].partition_broadcast(128).opt())
    nc.gpsimd.dma_start(out=b1sb[:], in_=b1t[:, :])
    nc.gpsimd.dma_start(out=b2b[:], in_=b2r[0:1, :].partition_broadcast(128).opt())
    ip = persist.tile([128, 1], i32, tag="ip")
    nc.gpsimd.iota(ip[:], pattern=[[0, 1]], base=0, channel_multiplier=1)
    nc.vector.tensor_copy(out=iotap[:], in_=ip[:])

    # ============ Phase R: sharded router (own 256 tokens, fp32) ============
    with (tc.tile_pool(name="r_x", bufs=2) as r_x,
          tc.tile_pool(name="r_ps", bufs=2, space="PSUM") as r_ps,
          tc.tile_pool(name="r_sb", bufs=2) as r_sb):
        for tt2 in range(2):
            xt_t = r_x.tile([128, H], f32, tag="xt_t")
            for qq in range(2):
                eng = nc.sync if qq == 0 else nc.scalar
                eng.dma_start(
                    out=xt_t[:, qq * (H // 2):(qq + 1) * (H // 2)],
                    in_=xtm[tt2, :, qq * (H // 2):(qq + 1) * (H // 2)])
            ps = r_ps.tile([128, E], f32, tag="r_ps")
            for hc in range(HT):
                nc.tensor.matmul(
                    out=ps[:], lhsT=xt_t[:, hc * 128:(hc + 1) * 128],
                    rhs=rws[:, hc * E:(hc + 1) * E],
                    start=(hc == 0), stop=(hc == HT - 1))
            lsb = r_sb.tile([128, E], f32, tag="lsb")
            nc.vector.tensor_copy(out=lsb[:], in_=ps[:])
            mx = r_sb.tile([128, 1], f32, tag="mx")
            nc.vector.tensor_reduce(out=mx[:], in_=lsb[:], op=AOP.max, axis=AX.X)
            nm = r_sb.tile([128, 1], f32, tag="nm")
            nc.vector.tensor_scalar_mul(nm[:], mx[:], -1.0)
            ex = r_sb.tile([128, E], f32, tag="ex")
            ssum = r_sb.tile([128, 1], f32, tag="ssum")
            nc.scalar.activation(out=ex[:], in_=lsb[:], func=AFT.Exp,
                                 bias=nm[:], scale=1.0, accum_out=ssum[:])
            rcp = r_sb.tile([128, 1], f32, tag="rcp")
            nc.vector.reciprocal(out=rcp[:], in_=ssum[:])
            pr = r_sb.tile([128, E], f32, tag="pr")
            nc.vector.tensor_scalar_mul(pr[:], ex[:], rcp[:])
            mx8 = r_sb.tile([128, 8], f32, tag="mx8")
            ix8 = r_sb.tile([128, 8], mybir.dt.uint32, tag="ix8")
            nc.vector.max_with_indices(out_max=mx8[:], out_indices=ix8[:],
                                       in_=pr[:])
            rv = r_sb.tile([128, 4], f32, tag="rv")
            nc.vector.tensor_copy(out=rv[:, 0:1], in_=ix8[:, 0:1])
            nc.vector.tensor_copy(out=rv[:, 1:2], in_=ix8[:, 1:2])
            nc.vector.tensor_copy(out=rv[:, 2:3], in_=mx8[:, 0:1])
            nc.vector.tensor_copy(out=rv[:, 3:4], in_=mx8[:, 1:2])
            nc.gpsimd.dma_start(out=rloc[tt2 * 128:(tt2 + 1) * 128, :], in_=rv[:])
        nc.gpsimd.collective_compute(
            "AllGather", AOP.bypass,
            replica_groups=RG,
            ins=[rloc[:, :].opt()],
            outs=[rall[:, :].opt()])

    # contiguous per-partition load of the gathered routing, (p, b) layout
    rb = persist.tile([128, 16 * 4], f32, tag="rb")
    nc.gpsimd.dma_start(
        out=rb[:], in_=rall[:, :].rearrange("(p b) c -> p (b c)", p=128))
    rbv = rb[:].rearrange("p (b c) -> p b c", c=4)
    e0a = persist.tile([128, BI], f32, tag="e0a")
    e1a = persist.tile([128, BI], f32, tag="e1a")
    p0a = persist.tile([128, BI], f32, tag="p0a")
    p1a = persist.tile([128, BI], f32, tag="p1a")
    nc.vector.tensor_copy(out=e0a[:], in_=rbv[:, :, 0].opt())
    nc.vector.tensor_copy(out=e1a[:], in_=rbv[:, :, 1].opt())
    nc.vector.tensor_copy(out=p0a[:], in_=rbv[:, :, 2].opt())
    nc.vector.tensor_copy(out=p1a[:], in_=rbv[:, :, 3].opt())

    # ============ Phase S: one-hot + 4-way segmented scan (fp16) ============
    # pack expert ids, roundtrip through DRAM to get the (k, t)-ordered
    # row (t = p*16 + b), then a broadcast load into [32, 1024]: partition
    # (e, seg) scans its 1024-long segment; segment offsets fixed up via a
    # small triangular matmul over the per-segment totals (issued after F1
    # on the tensor queue; all deps are ready long before it reaches PE).
    ip32 = sc.tile([32, 1], i32, tag="ip32")
    ip32f = sc.tile([32, 1], f32, tag="ip32f")
    nc.gpsimd.iota(ip32[:], pattern=[[0, 1]], base=0, channel_multiplier=1)
    nc.vector.tensor_copy(out=ip32f[:], in_=ip32[:])
    eri = sc.tile([1, 32], i32, tag="eri")
    nc.gpsimd.iota(eri[:], pattern=[[1, E], [0, NSEG]], base=0,
                   channel_multiplier=0)
    erf = sc.tile([1, 32], f32, tag="erf")
    nc.vector.tensor_copy(out=erf[:], in_=eri[:])
    nc.gpsimd.dma_start(out=e32d[0:1, :], in_=erf[:])
    eidx = sc.tile([32, 1], f32, tag="eidx")
    nc.gpsimd.dma_start(
        out=eidx[:, :],
        in_=e32d[0:1, :].rearrange("a (c u) -> (a c) u", u=1))
    sidx = sc.tile([32, 1], f32, tag="sidx")
    nc.vector.scalar_tensor_tensor(out=sidx[:], in0=eidx[:],
                                   scalar=-float(NSEG), in1=ip32f[:],
                                   op0=AOP.mult, op1=AOP.add)
    # Mt[p', p] = same expert and seg(p') < seg(p): exclusive prefix mask
    jmi = sc.tile([32, 32], i32, tag="jmi")
    nc.gpsimd.iota(jmi[:], pattern=[[0, E], [1, NSEG]], base=0,
                   channel_multiplier=0)
    jm = sc.tile([32, 32], f32, tag="jm")
    nc.vector.tensor_copy(out=jm[:], in_=jmi[:])
    eci = sc.tile([32, 32], i32, tag="eci")
    nc.gpsimd.iota(eci[:], pattern=[[1, E], [0, NSEG]], base=0,
                   channel_multiplier=0)
    ec = sc.tile([32, 32], f32, tag="ec")
    nc.vector.tensor_copy(out=ec[:], in_=eci[:])
    Mt = sc.tile([32, 32], f16, tag="Mt")
    me32 = sc.tile([32, 32], f32, tag="me32")
    nc.vector.tensor_scalar(out=me32[:], in0=ec[:],
                            scalar1=eidx[:], scalar2=None, op0=AOP.is_equal)
    ms32 = sc.tile([32, 32], f32, tag="ms32")
    nc.vector.tensor_scalar(out=ms32[:], in0=jm[:],
                            scalar1=sidx[:], scalar2=None, op0=AOP.is_gt)
    nc.vector.tensor_tensor(out=Mt[:], in0=me32[:], in1=ms32[:],
                            op=AOP.mult)
    sel4 = sc.tile([32, NSEG], f16, tag="sel4")
    iseg = sc.tile([32, NSEG], i32, tag="iseg")
    nc.gpsimd.iota(iseg[:], pattern=[[1, NSEG]], base=0, channel_multiplier=0)
    isegf = sc.tile([32, NSEG], f32, tag="isegf")
    nc.vector.tensor_copy(out=isegf[:], in_=iseg[:])
    nc.vector.tensor_scalar(out=sel4[:], in0=isegf[:], scalar1=sidx[:],
                            scalar2=None, op0=AOP.is_equal)

    e01 = sc.tile([128, 32], f32, tag="e01")
    nc.vector.tensor_copy(out=e01[:, 0:16], in_=e0a[:])
    nc.vector.tensor_copy(out=e01[:, 16:32], in_=e1a[:])
    for e in range(E):
        nc.gpsimd.dma_start(
            out=ebuf8[e:e + 1, :].rearrange(
                "a (k p b) -> (a p) k b", k=2, p=128),
            in_=e01[:].rearrange("p (k b) -> p k b", k=2))
    ohsrc = sc.tile([32, SEGL], f32, tag="ohsrc")
    nc.gpsimd.dma_start(
        out=ohsrc[:],
        in_=ebuf8[:, :].rearrange("e (s c) -> (e s) c", s=NSEG))
    ohcat = sc.tile([32, SEGL], f16, tag="ohcat")
    nc.vector.tensor_scalar(out=ohcat[:], in0=ohsrc[:], scalar1=eidx[:],
                            scalar2=None, op0=AOP.is_equal)
    ones2n = sc.tile([32, SEGL], f16, tag="ones2n")
    nc.vector.memset(ones2n[:], 1.0)
    cum = sc.tile([32, SEGL], f16, tag="cum")
    nc.vector.tensor_tensor_scan(out=cum[:], data0=ones2n[:], data1=ohcat[:],
                                 initial=0.0, op0=AOP.mult, op1=AOP.add)
    tot32 = sc.tile([32, 1], f16, tag="tot32")
    with nc.allow_low_precision(reason="segment counts <= 1024, f16-exact"):
        nc.vector.tensor_reduce(out=tot32[:], in_=ohcat[:], op=AOP.add,
                                axis=AX.X)

    # ============ Phase F1 (dense, 512 pair tokens, local F half) ============
    with tc.tile_pool(name="g", bufs=1) as g_pool:
        g = []
        with (tc.tile_pool(name="f1_w", bufs=4) as f1_w,
              tc.tile_pool(name="f1_ps", bufs=2, space="PSUM") as f1_ps):
            for ft in range(FTH):
                w1_t = f1_w.tile([128, HT * 128], bf16, tag="w1_t")
                QW = HT * 128 // 2
                for qq in range(2):
                    eng = nc.sync if qq == 0 else nc.scalar
                    eng.dma_start(
                        out=w1_t[:, qq * QW:(qq + 1) * QW],
                        in_=w1T[ft, :, qq * QW:(qq + 1) * QW])
                ps = f1_ps.tile([128, TOKP], f32, tag="f1_ps")
                for hc in range(HT):
                    nc.tensor.matmul(
                        out=ps[:], lhsT=w1_t[:, hc * 128:(hc + 1) * 128],
                        rhs=xcT[:, hc * TOKP:(hc + 1) * TOKP],
                        start=(hc == 0), stop=(hc == HT - 1))
                gt = g_pool.tile([128, TOKP], bf16, tag=f"g_{ft}")
                nc.scalar.activation(out=gt[:], in_=ps[:], func=AFT.Gelu,
                                     bias=b1sb[:, ft:ft + 1], scale=1.0)
                g.append(gt)

        # ---- scan fixup (tensor) + per-token weight chain ----
        with tc.tile_pool(name="s_ps", bufs=2, space="PSUM") as s_ps:
            offp = s_ps.tile([32, 1], f32, tag="offp")
            nc.tensor.matmul(out=offp[:], lhsT=Mt[:], rhs=tot32[:],
                             start=True, stop=True)
            off32 = sc.tile([32, 1], f32, tag="off32")
            nc.vector.tensor_copy(out=off32[:], in_=offp[:])
            cumf = sc.tile([32, SEGL], f16, tag="cumf")
            nc.vector.tensor_scalar(out=cumf[:], in0=cum[:], scalar1=off32[:],
                                    scalar2=None, op0=AOP.add)
            ohcum = sc.tile([32, SEGL], f16, tag="ohcum")
            nc.vector.tensor_tensor(out=ohcum[:], in0=ohcat[:], in1=cumf[:],
                                    op=AOP.mult)
            posrow = sc.tile([1, 2 * N], f32, tag="posrow")
            for s in range(NSEG):
                for ch in range(SEGL // 512):
                    pps = s_ps.tile([1, 512], f32, tag="pps")
                    nc.tensor.matmul(out=pps[:], lhsT=sel4[:, s:s + 1],
                                     rhs=ohcum[:, ch * 512:(ch + 1) * 512],
                                     start=True, stop=True)
                    nc.vector.tensor_scalar_add(
                        posrow[:, s * SEGL + ch * 512:s * SEGL + (ch + 1) * 512],
                        pps[:], -1.0)
            nc.gpsimd.dma_start(out=posd[:, 0:N], in_=posrow[:, 0:N])
            nc.gpsimd.dma_start(out=posd[:, N:2 * N], in_=posrow[:, N:2 * N])

            # w(t) = p0*(pos0<CAP) + p1*(pos1<CAP) for all tokens -> wd,
            # then indirect-gather own 256 into wt[128, 2] (col = tok//128)
            pos0a = sc.tile([128, BI], f32, tag="pos0a")
            pos1a = sc.tile([128, BI], f32, tag="pos1a")
            nc.gpsimd.dma_start(
                out=pos0a[:],
                in_=posd[0:1, 0:N].rearrange("a (p b) -> (a p) b", p=128))
            nc.gpsimd.dma_start(
                out=pos1a[:],
                in_=posd[0:1, N:2 * N].rearrange("a (p b) -> (a p) b", p=128))
            wall = sc.tile([128, BI], f32, tag="wall")
            k0 = sc.tile([128, BI], f32, tag="k0")
            nc.vector.tensor_scalar(out=k0[:], in0=pos0a[:], scalar1=float(CAP),
                                    scalar2=None, op0=AOP.is_lt)
            nc.vector.tensor_tensor(out=k0[:], in0=k0[:], in1=p0a[:], op=AOP.mult)
            k1 = sc.tile([128, BI], f32, tag="k1")
            nc.vector.tensor_scalar(out=k1[:], in0=pos1a[:], scalar1=float(CAP),
                                    scalar2=None, op0=AOP.is_lt)
            nc.vector.tensor_tensor(out=k1[:], in0=k1[:], in1=p1a[:], op=AOP.mult)
            nc.vector.tensor_tensor(out=wall[:], in0=k0[:], in1=k1[:], op=AOP.add)
            nc.gpsimd.dma_start(
                out=wd[:, :].rearrange("(p b) a -> p (b a)", p=128), in_=wall[:])
            wif = sc.tile([128, 2], f32, tag="wif")
            nc.vector.scalar_tensor_tensor(out=wif[:, 0:1], in0=cidb[:],
                                           scalar=float(TOKC), in1=iotap[:],
                                           op0=AOP.mult, op1=AOP.add)
            nc.vector.tensor_scalar_add(wif[:, 1:2], wif[:, 0:1], 128.0)
            nc.vector.tensor_copy(out=widx[:], in_=wif[:])
            for q in range(2):
                nc.gpsimd.indirect_dma_start(
                    out=wt[:, q:q + 1], out_offset=None,
                    in_=wd[:, :],
                    in_offset=bass.IndirectOffsetOnAxis(
                        ap=widx[:, q:q + 1], axis=0))

        # ============ Phase F2 + pair ReduceScatter per H-chunk ============
        # consume (epilogue of chunk ci-1) runs one chunk behind so the
        # gpsimd queue never blocks on an in-flight ReduceScatter
        with (tc.tile_pool(name="f2_w", bufs=12) as f2_w,
              tc.tile_pool(name="f2_ps", bufs=2, space="PSUM") as f2_ps,
              tc.tile_pool(name="pqs", bufs=2) as pqs,
              tc.tile_pool(name="cbp", bufs=2) as cbp):

            def consume(ci, off, w):
                rq = cbp.tile([128, 2 * 512], bf16, tag="rq")
                nc.gpsimd.dma_start(
                    out=rq[:, :2 * w].rearrange("p (q f) -> p q f", q=2),
                    in_=arq[ci][:, :].rearrange("(q p) f -> p q f", p=128))
                for q in range(2):
                    hs = cbp.tile([128, 512], f32, tag="hs")
                    nc.vector.tensor_tensor(
                        out=hs[:, :w], in0=rq[:, q * w:(q + 1) * w],
                        in1=b2b[:, off:off + w], op=AOP.add)
                    o = cbp.tile([128, 512], f32, tag="o")
                    nc.vector.tensor_scalar_mul(
                        o[:, :w], hs[:, :w], wt[:, q:q + 1])
                    nc.gpsimd.dma_start(
                        out=out[q * 128:(q + 1) * 128, off:off + w],
                        in_=o[:, :w])

            for ci, (off, w) in enumerate(CHUNKS):
                psq = []
                for tt in range(4):
                    psq_t = f2_ps.tile([128, 512], f32, tag=f"f2_ps_{tt}")
                    psq.append(psq_t)
                for fc in range(FTH):
                    w2_t = f2_w.tile([128, 512], bf16, tag="w2_t")
                    eng = nc.sync if (fc % 2 == 0) else nc.scalar
                    eng.dma_start(out=w2_t[:, :w],
                                  in_=w2q[fc, :, off:off + w])
                    for tt in range(4):
                        nc.tensor.matmul(
                            out=psq[tt][:, :w],
                            lhsT=g[fc][:, tt * 128:(tt + 1) * 128],
                            rhs=w2_t[:, :w],
                            start=(fc == 0), stop=(fc == FTH - 1))
                for tt in range(4):
                    pb = pqs.tile([128, 512], bf16, tag=f"pb{tt % 2}")
                    nc.vector.tensor_copy(out=pb[:, :w], in_=psq[tt][:, :w])
                    nc.gpsimd.dma_start(
                        out=pq[ci][tt * 128:(tt + 1) * 128, :], in_=pb[:, :w])
                nc.gpsimd.collective_compute(
                    "ReduceScatter", AOP.add, replica_groups=PRG,
                    ins=[pq[ci][:, :].opt()],
                    outs=[arq[ci][:, :].opt()])
                if ci > 0:
                    consume(ci - 1, *CHUNKS[ci - 1])
            consume(len(CHUNKS) - 1, *CHUNKS[-1])


# ======================== host-side glue ========================

_CACHE = {}


def _prep_inputs(hidden_states, router_w, w1, b1, w2, b2):
    x = np.asarray(hidden_states, np.float32).reshape(-1, H)
    xT = np.ascontiguousarray(x.T)
    w1Tm = np.asarray(w1, np.float32).T.astype(ml_dtypes.bfloat16)
    w2Tm = np.asarray(w2, np.float32).T.astype(ml_dtypes.bfloat16)
    w1tt = np.ascontiguousarray(
        w1Tm.reshape(16, 128, 64, 128).transpose(2, 1, 0, 3)).reshape(64, 128, 2048)
    w2qm = np.ascontiguousarray(w2Tm.reshape(64, 128, 2048))
    b1tf = np.ascontiguousarray(np.asarray(b1, np.float32).reshape(64, 128).T)
    base = {
        "rwT": np.ascontiguousarray(
            np.asarray(router_w, np.float32).T.reshape(16, 128, 8)
            .transpose(1, 0, 2).reshape(128, 128)),
        "b2r": np.asarray(b2, np.float32).reshape(1, H),
    }
    xtmf = np.ascontiguousarray(
        xT.reshape(16, 128, 16, 128).transpose(2, 1, 0, 3)).reshape(16, 128, 2048)
    xTb = xT.astype(ml_dtypes.bfloat16)
    ins = []
    for c in range(NCORES):
        hh = c % 2          # ffn half
        p = c // 2          # pair (owns tokens [512p, 512p+512))
        m = dict(base)
        m["w1tt"] = np.ascontiguousarray(w1tt[hh * FTH:(hh + 1) * FTH])
        m["w2q"] = np.ascontiguousarray(w2qm[hh * FTH:(hh + 1) * FTH])
        m["b1t"] = np.ascontiguousarray(b1tf[:, hh * FTH:(hh + 1) * FTH])
        m["xtm"] = np.ascontiguousarray(xtmf[2 * c:2 * c + 2])
        # xtt[pp, ht*512 + t] = x[512p + t, ht*128 + pp]
        m["xtt"] = np.ascontiguousarray(
            xTb[:, p * TOKP:(p + 1) * TOKP].reshape(HT, 128, TOKP)
            .transpose(1, 0, 2).reshape(128, HT * TOKP))
        m["cid"] = np.full((1, 1), float(c), np.float32)
        ins.append(m)
    return ins


def _get_nc():
    if "nc" not in _CACHE:
        apply()  # tile drain patch
        nc = bass.Bass(num_devices=NCORES)
        build_moe(nc)
        split_multi_waits(nc)
        _CACHE["nc"] = nc
    return _CACHE["nc"]


def kernel(hidden_states, router_w, w1, b1, w2, b2):
    from concourse.bass_utils import run_bass_kernel_spmd

    orig_shape = np.asarray(hidden_states).shape
    nc = _get_nc()
    ins = _prep_inputs(hidden_states, router_w, w1, b1, w2, b2)
    res = run_bass_kernel_spmd(nc, ins, core_ids=list(range(NCORES)))
    full = np.concatenate([res.results[c]["out"] for c in range(NCORES)], axis=0)
    return full.reshape(orig_shape).astype(np.float32)


# revision 21
# speedup vs baseline: 1.1743x; 1.1743x over previous
"""Trainium2 8-core MoE layer kernel (token x ffn sharded dense FFN, Bass/Tile).

Contract: kernel(**inputs) takes the full unsharded numpy inputs of the
MoE reference (hidden_states, router_w, w1, b1, w2, b2) and returns the
full [2, 1024, 2048] float32 output.

Key identity: the reference's experts all share one FFN (w1/b1/w2/b2 are
not per-expert), so for every kept (token, k) slot the expert output is
FFN(x[t]) and the combine collapses to
    out[t] = (sum_k kept_k(t) * gate_k(t)) * FFN(x[t]).
Routing therefore only determines a per-token scalar; the FFN itself is
a dense [N, H] pass.

Sharding: cores form pairs (2p, 2p+1) owning tokens [512p, 512p+512).
Within a pair the ffn dim F is split in half (4096 per core), so each
core streams only 32 MB of weights (the kernel is HBM-bound at ~220 GB/s
per core, while the PE floor is ~134 us per GEMM phase). Each core
computes partial y = gelu(x W1h^T + b1h) W2h^T for all 512 pair tokens;
a per-H-chunk bf16 ReduceScatter(add) over the pair then hands every
core the finished rows of its own 256 tokens (= global tokens
[256c, 256c+256), matching the host-side concat).

The fp32 router runs on each core's own 256 tokens; an AllGather shares
the [N, 4] routing decisions; a replicated capacity scan produces the
per-token weight w(t), indirect-gathered per core. That whole chain
rides on vector/gpsimd during F1; its two tiny matmul groups issue
between F1 and F2 on the tensor queue.
"""
import numpy as np
import ml_dtypes

import concourse.bass as bass
import concourse.mybir as mybir
import concourse.tile as tile

_PATCH_DOC = """Patch TileContext._drain_and_barrier: the stock version stuffs every
outstanding semaphore wait onto one SP Drain instruction; the installed
walrus rejects >1 sync wait per non-EventSemaphore instruction
("Too many sync wait commands"). Split the waits across a chain of SP
nops, then drain/barrier as before."""
import concourse.tile as tile_mod
from concourse.vector_clock import ScopedClock


def _patched_drain_and_barrier(self, tick_clock, wait_clock):
    nc = self.nc
    carrier = nc.sync.nop(nofuse=True, hint="drain_wait_carrier")
    wait_clock.add_sem_waits(
        carrier.ins, ScopedClock({None: tick_clock.global_clock})
    )
    waits = list(carrier.ins.sync_info.on_wait)
    if len(waits) > 1:
        carrier.ins.sync_info.on_wait = waits[:1]
        import bass_rust as _br
        for w in waits[1:]:
            extra = nc.sync.nop(nofuse=True, hint="drain_wait_carrier")
            extra.ins.sync_info = _br.SyncInfo(on_wait=[w], on_update=[])

    nc.sync.drain()
    nc.all_engine_barrier()
    assert self.sems is not None
    popped = nc._tile_sem_poison_stack.pop()
    assert popped is self._sem_poison
    nc.clear_and_free_semaphores(list(self.sems.allocated().values()))
    nc.all_engine_barrier()


def apply():
    tile_mod.TileContext._drain_and_barrier = _patched_drain_and_barrier


import concourse.mybir as mybir
import bass_rust as _br


def split_multi_waits(nc):
    """Walrus in this container accepts at most ONE sync wait per
    instruction. Hoist extra waits onto same-engine NoOps inserted
    immediately before the offending instruction."""
    ctr = 0
    for f in nc.m.functions:
        for b in f.blocks:
            insts = b.instructions
            need = any(
                inst.sync_info is not None and len(inst.sync_info.on_wait) > 1
                for inst in insts
            )
            if not need:
                continue
            out = []
            for inst in insts:
                si = inst.sync_info
                if si is not None and len(si.on_wait) > 1:
                    waits = list(si.on_wait)
                    for w in waits[:-1]:
                        nop = mybir.InstNoOp(name=f"I-wsplit-{ctr}", ins=[], outs=[])
                        ctr += 1
                        nop.engine = inst.engine
                        nop.sync_info = _br.SyncInfo(on_wait=[w], on_update=[])
                        out.append(nop)
                    si.on_wait = waits[-1:]
                out.append(inst)
            b.instructions = out
    return ctr


E, TOPK, CAP, H, F, N, NCORES = 8, 2, 512, 2048, 8192, 2048, 8
HT = H // 128                 # 16 hidden tiles
FH = F // 2                   # 4096 ffn columns per core
FTH = FH // 128               # 32 local ffn tiles
TOKC = N // NCORES            # 256 output tokens per core
TOKP = 2 * TOKC               # 512 tokens per pair
BI = N // 128                 # 16
NSEG = 4
SEGL = 2 * N // NSEG
# F2 H-chunks; the small last chunk keeps the exposed final
# ReduceScatter + epilogue short
CHUNKS = [(0, 512), (512, 512), (1024, 512), (1536, 384), (1920, 128)]

f32 = mybir.dt.float32
f16 = mybir.dt.float16
bf16 = mybir.dt.bfloat16
i32 = mybir.dt.int32
AOP = mybir.AluOpType
AFT = mybir.ActivationFunctionType
AX = mybir.AxisListType


def build_moe(nc: bass.Bass):
    xtm = nc.dram_tensor("xtm", [2, 128, H], f32, kind="ExternalInput")
    rk = nc.dram_tensor("rk", [1, 1], f32, kind="ExternalInput")
    xtt = nc.dram_tensor("xtt", [128, HT * TOKP], bf16, kind="ExternalInput")
    rwT = nc.dram_tensor("rwT", [128, HT * E], f32, kind="ExternalInput")
    w1T = nc.dram_tensor("w1tt", [FTH, 128, HT * 128], bf16, kind="ExternalInput")
    w2q = nc.dram_tensor("w2q", [FTH, 128, H], bf16, kind="ExternalInput")
    b1t = nc.dram_tensor("b1t", [128, FTH], f32, kind="ExternalInput")
    b2r = nc.dram_tensor("b2r", [1, H], f32, kind="ExternalInput")
    cid = nc.dram_tensor("cid", [1, 1], f32, kind="ExternalInput")
    out = nc.dram_tensor("out", [TOKC, H], f32, kind="ExternalOutput")

    rloc = nc.dram_tensor("rloc", [TOKC, 4], f32)
    rall = nc.dram_tensor("rall", [N, 4], f32, addr_space="Shared")
    ebuf8 = nc.dram_tensor("ebuf8", [E, 2 * N], f32)
    e32d = nc.dram_tensor("e32d", [1, 32], f32)
    posd = nc.dram_tensor("posd", [1, 2 * N], f32)
    wd = nc.dram_tensor("wd", [N, 1], f32)
    pq = [nc.dram_tensor(f"pq{ci}", [TOKP, w], bf16)
          for ci, (o, w) in enumerate(CHUNKS)]
    arq = [nc.dram_tensor(f"arq{ci}", [2 * TOKP, w], bf16)
           for ci, (o, w) in enumerate(CHUNKS)]

    with tile.TileContext(nc, num_cores=NCORES) as tc:
        with tc.tile_pool(name="persist", bufs=1) as persist:
            _body(nc, tc, persist, xtm, rk, xtt, rwT, w1T, w2q, b1t, b2r,
                  cid, out, rloc, rall, ebuf8, e32d, posd, wd, pq, arq)
    return nc


def _body(nc, tc, persist, xtm, rk, xtt, rwT, w1T, w2q, b1t, b2r, cid, out,
          rloc, rall, ebuf8, e32d, posd, wd, pq, arq):
    RG = [list(range(NCORES))]
    PRG = [[2 * p, 2 * p + 1] for p in range(NCORES // 2)]
    sc = persist

    # ---- persistent tiles ----
    b2b = persist.tile([128, H], f32, tag="b2b")
    cidb = persist.tile([128, 1], f32, tag="cidb")
    b1sb = persist.tile([128, FTH], f32, tag="b1sb")
    rws = persist.tile([128, HT * E], f32, tag="rws")
    xcT = persist.tile([128, HT * TOKP], bf16, tag="xcT")
    iotap = persist.tile([128, 1], f32, tag="iotap")
    rkb = persist.tile([128, 1], f32, tag="rkb")
    wt = persist.tile([128, 2], f32, tag="wt")
    widx = persist.tile([128, 2], i32, tag="widx")
    # consume-gather row indices into arq[ci] ([2*TOKP, w], two 512-row
    # partial blocks): summand A = my block's rows for my tokens
    # (rank*512 + rank*256 + q*128 + p), summand B = partner block's rows
    # for my tokens ((1-rank)*512 + rank*256 + q*128 + p)
    wiA = persist.tile([128, 2], i32, tag="wiA")
    wiB = persist.tile([128, 2], i32, tag="wiB")

    # router x first, then F1 x, split across the two fast queues so the
    # router and F1 start as early as possible; everything else on gpsimd
    QX = HT * TOKP // 2
    nc.gpsimd.dma_start(out=rws[:], in_=rwT[:, :])
    nc.gpsimd.dma_start(out=cidb[:], in_=cid[0:1, :].partition_broadcast(128).opt())
    nc.gpsimd.dma_start(out=rkb[:], in_=rk[0:1, :].partition_broadcast(128).opt())
    nc.gpsimd.dma_start(out=b1sb[:], in_=b1t[:, :])
    nc.gpsimd.dma_start(out=b2b[:], in_=b2r[0:1, :].partition_broadcast(128).opt())
    ip = persist.tile([128, 1], i32, tag="ip")
    nc.gpsimd.iota(ip[:], pattern=[[0, 1]], base=0, channel_multiplier=1)
    nc.vector.tensor_copy(out=iotap[:], in_=ip[:])

    # ============ Phase R: sharded router (own 256 tokens, fp32) ============
    with (tc.tile_pool(name="r_x", bufs=2) as r_x,
          tc.tile_pool(name="r_ps", bufs=2, space="PSUM") as r_ps,
          tc.tile_pool(name="r_sb", bufs=2) as r_sb):
        for tt2 in range(2):
            xt_t = r_x.tile([128, H], f32, tag="xt_t")
            for qq in range(2):
                eng = nc.sync if qq == 0 else nc.scalar
                eng.dma_start(
                    out=xt_t[:, qq * (H // 2):(qq + 1) * (H // 2)],
                    in_=xtm[tt2, :, qq * (H // 2):(qq + 1) * (H // 2)])
            ps = r_ps.tile([128, E], f32, tag="r_ps")
            for hc in range(HT):
                nc.tensor.matmul(
                    out=ps[:], lhsT=xt_t[:, hc * 128:(hc + 1) * 128],
                    rhs=rws[:, hc * E:(hc + 1) * E],
                    start=(hc == 0), stop=(hc == HT - 1))
            lsb = r_sb.tile([128, E], f32, tag="lsb")
            nc.vector.tensor_copy(out=lsb[:], in_=ps[:])
            mx = r_sb.tile([128, 1], f32, tag="mx")
            nc.vector.tensor_reduce(out=mx[:], in_=lsb[:], op=AOP.max, axis=AX.X)
            nm = r_sb.tile([128, 1], f32, tag="nm")
            nc.vector.tensor_scalar_mul(nm[:], mx[:], -1.0)
            ex = r_sb.tile([128, E], f32, tag="ex")
            ssum = r_sb.tile([128, 1], f32, tag="ssum")
            nc.scalar.activation(out=ex[:], in_=lsb[:], func=AFT.Exp,
                                 bias=nm[:], scale=1.0, accum_out=ssum[:])
            rcp = r_sb.tile([128, 1], f32, tag="rcp")
            nc.vector.reciprocal(out=rcp[:], in_=ssum[:])
            pr = r_sb.tile([128, E], f32, tag="pr")
            nc.vector.tensor_scalar_mul(pr[:], ex[:], rcp[:])
            mx8 = r_sb.tile([128, 8], f32, tag="mx8")
            ix8 = r_sb.tile([128, 8], mybir.dt.uint32, tag="ix8")
            nc.vector.max_with_indices(out_max=mx8[:], out_indices=ix8[:],
                                       in_=pr[:])
            rv = r_sb.tile([128, 4], f32, tag="rv")
            nc.vector.tensor_copy(out=rv[:, 0:1], in_=ix8[:, 0:1])
            nc.vector.tensor_copy(out=rv[:, 1:2], in_=ix8[:, 1:2])
            nc.vector.tensor_copy(out=rv[:, 2:3], in_=mx8[:, 0:1])
            nc.vector.tensor_copy(out=rv[:, 3:4], in_=mx8[:, 1:2])
            nc.gpsimd.dma_start(out=rloc[tt2 * 128:(tt2 + 1) * 128, :], in_=rv[:])
        nc.sync.dma_start(out=xcT[:, 0:QX], in_=xtt[:, 0:QX])
        nc.scalar.dma_start(out=xcT[:, QX:2 * QX], in_=xtt[:, QX:2 * QX])
        nc.gpsimd.collective_compute(
            "AllGather", AOP.bypass,
            replica_groups=RG,
            ins=[rloc[:, :].opt()],
            outs=[rall[:, :].opt()])

    # contiguous per-partition load of the gathered routing, (p, b) layout
    rb = persist.tile([128, 16 * 4], f32, tag="rb")
    nc.gpsimd.dma_start(
        out=rb[:], in_=rall[:, :].rearrange("(p b) c -> p (b c)", p=128))
    rbv = rb[:].rearrange("p (b c) -> p b c", c=4)
    e0a = persist.tile([128, BI], f32, tag="e0a")
    e1a = persist.tile([128, BI], f32, tag="e1a")
    p0a = persist.tile([128, BI], f32, tag="p0a")
    p1a = persist.tile([128, BI], f32, tag="p1a")
    nc.vector.tensor_copy(out=e0a[:], in_=rbv[:, :, 0].opt())
    nc.vector.tensor_copy(out=e1a[:], in_=rbv[:, :, 1].opt())
    nc.vector.tensor_copy(out=p0a[:], in_=rbv[:, :, 2].opt())
    nc.vector.tensor_copy(out=p1a[:], in_=rbv[:, :, 3].opt())

    # ============ Phase S: one-hot + 4-way segmented scan (fp16) ============
    # pack expert ids, roundtrip through DRAM to get the (k, t)-ordered
    # row (t = p*16 + b), then a broadcast load into [32, 1024]: partition
    # (e, seg) scans its 1024-long segment; segment offsets fixed up via a
    # small triangular matmul over the per-segment totals (issued after F1
    # on the tensor queue; all deps are ready long before it reaches PE).
    ip32 = sc.tile([32, 1], i32, tag="ip32")
    ip32f = sc.tile([32, 1], f32, tag="ip32f")
    nc.gpsimd.iota(ip32[:], pattern=[[0, 1]], base=0, channel_multiplier=1)
    nc.vector.tensor_copy(out=ip32f[:], in_=ip32[:])
    eri = sc.tile([1, 32], i32, tag="eri")
    nc.gpsimd.iota(eri[:], pattern=[[1, E], [0, NSEG]], base=0,
                   channel_multiplier=0)
    erf = sc.tile([1, 32], f32, tag="erf")
    nc.vector.tensor_copy(out=erf[:], in_=eri[:])
    nc.gpsimd.dma_start(out=e32d[0:1, :], in_=erf[:])
    eidx = sc.tile([32, 1], f32, tag="eidx")
    nc.gpsimd.dma_start(
        out=eidx[:, :],
        in_=e32d[0:1, :].rearrange("a (c u) -> (a c) u", u=1))
    sidx = sc.tile([32, 1], f32, tag="sidx")
    nc.vector.scalar_tensor_tensor(out=sidx[:], in0=eidx[:],
                                   scalar=-float(NSEG), in1=ip32f[:],
                                   op0=AOP.mult, op1=AOP.add)
    # Mt[p', p] = same expert and seg(p') < seg(p): exclusive prefix mask
    jmi = sc.tile([32, 32], i32, tag="jmi")
    nc.gpsimd.iota(jmi[:], pattern=[[0, E], [1, NSEG]], base=0,
                   channel_multiplier=0)
    jm = sc.tile([32, 32], f32, tag="jm")
    nc.vector.tensor_copy(out=jm[:], in_=jmi[:])
    eci = sc.tile([32, 32], i32, tag="eci")
    nc.gpsimd.iota(eci[:], pattern=[[1, E], [0, NSEG]], base=0,
                   channel_multiplier=0)
    ec = sc.tile([32, 32], f32, tag="ec")
    nc.vector.tensor_copy(out=ec[:], in_=eci[:])
    Mt = sc.tile([32, 32], f16, tag="Mt")
    me32 = sc.tile([32, 32], f32, tag="me32")
    nc.vector.tensor_scalar(out=me32[:], in0=ec[:],
                            scalar1=eidx[:], scalar2=None, op0=AOP.is_equal)
    ms32 = sc.tile([32, 32], f32, tag="ms32")
    nc.vector.tensor_scalar(out=ms32[:], in0=jm[:],
                            scalar1=sidx[:], scalar2=None, op0=AOP.is_gt)
    nc.vector.tensor_tensor(out=Mt[:], in0=me32[:], in1=ms32[:],
                            op=AOP.mult)
    sel4 = sc.tile([32, NSEG], f16, tag="sel4")
    iseg = sc.tile([32, NSEG], i32, tag="iseg")
    nc.gpsimd.iota(iseg[:], pattern=[[1, NSEG]], base=0, channel_multiplier=0)
    isegf = sc.tile([32, NSEG], f32, tag="isegf")
    nc.vector.tensor_copy(out=isegf[:], in_=iseg[:])
    nc.vector.tensor_scalar(out=sel4[:], in0=isegf[:], scalar1=sidx[:],
                            scalar2=None, op0=AOP.is_equal)

    e01 = sc.tile([128, 32], f32, tag="e01")
    nc.vector.tensor_copy(out=e01[:, 0:16], in_=e0a[:])
    nc.vector.tensor_copy(out=e01[:, 16:32], in_=e1a[:])
    for e in range(E):
        nc.gpsimd.dma_start(
            out=ebuf8[e:e + 1, :].rearrange(
                "a (k p b) -> (a p) k b", k=2, p=128),
            in_=e01[:].rearrange("p (k b) -> p k b", k=2))
    ohsrc = sc.tile([32, SEGL], f32, tag="ohsrc")
    nc.gpsimd.dma_start(
        out=ohsrc[:],
        in_=ebuf8[:, :].rearrange("e (s c) -> (e s) c", s=NSEG))
    ohcat = sc.tile([32, SEGL], f16, tag="ohcat")
    nc.vector.tensor_scalar(out=ohcat[:], in0=ohsrc[:], scalar1=eidx[:],
                            scalar2=None, op0=AOP.is_equal)
    ones2n = sc.tile([32, SEGL], f16, tag="ones2n")
    nc.vector.memset(ones2n[:], 1.0)
    cum = sc.tile([32, SEGL], f16, tag="cum")
    nc.vector.tensor_tensor_scan(out=cum[:], data0=ones2n[:], data1=ohcat[:],
                                 initial=0.0, op0=AOP.mult, op1=AOP.add)
    tot32 = sc.tile([32, 1], f16, tag="tot32")
    with nc.allow_low_precision(reason="segment counts <= 1024, f16-exact"):
        nc.vector.tensor_reduce(out=tot32[:], in_=ohcat[:], op=AOP.add,
                                axis=AX.X)

    # ============ Phase F1 (dense, 512 pair tokens, local F half) ============
    with tc.tile_pool(name="g", bufs=1) as g_pool:
        g = []
        with (tc.tile_pool(name="f1_w", bufs=4) as f1_w,
              tc.tile_pool(name="f1_ps", bufs=2, space="PSUM") as f1_ps):
            for ft in range(FTH):
                w1_t = f1_w.tile([128, HT * 128], bf16, tag="w1_t")
                QW = HT * 128 // 2
                for qq in range(2):
                    eng = nc.sync if qq == 0 else nc.scalar
                    eng.dma_start(
                        out=w1_t[:, qq * QW:(qq + 1) * QW],
                        in_=w1T[ft, :, qq * QW:(qq + 1) * QW])
                ps = f1_ps.tile([128, TOKP], f32, tag="f1_ps")
                for hc in range(HT):
                    nc.tensor.matmul(
                        out=ps[:], lhsT=w1_t[:, hc * 128:(hc + 1) * 128],
                        rhs=xcT[:, hc * TOKP:(hc + 1) * TOKP],
                        start=(hc == 0), stop=(hc == HT - 1))
                gt = g_pool.tile([128, TOKP], bf16, tag=f"g_{ft}")
                nc.scalar.activation(out=gt[:], in_=ps[:], func=AFT.Gelu,
                                     bias=b1sb[:, ft:ft + 1], scale=1.0)
                g.append(gt)

        # ---- scan fixup (tensor) + per-token weight chain ----
        with tc.tile_pool(name="s_ps", bufs=2, space="PSUM") as s_ps:
            offp = s_ps.tile([32, 1], f32, tag="offp")
            nc.tensor.matmul(out=offp[:], lhsT=Mt[:], rhs=tot32[:],
                             start=True, stop=True)
            off32 = sc.tile([32, 1], f32, tag="off32")
            nc.vector.tensor_copy(out=off32[:], in_=offp[:])
            cumf = sc.tile([32, SEGL], f16, tag="cumf")
            nc.vector.tensor_scalar(out=cumf[:], in0=cum[:], scalar1=off32[:],
                                    scalar2=None, op0=AOP.add)
            ohcum = sc.tile([32, SEGL], f16, tag="ohcum")
            nc.vector.tensor_tensor(out=ohcum[:], in0=ohcat[:], in1=cumf[:],
                                    op=AOP.mult)
            posrow = sc.tile([1, 2 * N], f32, tag="posrow")
            for s in range(NSEG):
                for ch in range(SEGL // 512):
                    pps = s_ps.tile([1, 512], f32, tag="pps")
                    nc.tensor.matmul(out=pps[:], lhsT=sel4[:, s:s + 1],
                                     rhs=ohcum[:, ch * 512:(ch + 1) * 512],
                                     start=True, stop=True)
                    nc.vector.tensor_scalar_add(
                        posrow[:, s * SEGL + ch * 512:s * SEGL + (ch + 1) * 512],
                        pps[:], -1.0)
            nc.gpsimd.dma_start(out=posd[:, 0:N], in_=posrow[:, 0:N])
            nc.gpsimd.dma_start(out=posd[:, N:2 * N], in_=posrow[:, N:2 * N])

            # w(t) = p0*(pos0<CAP) + p1*(pos1<CAP) for all tokens -> wd,
            # then indirect-gather own 256 into wt[128, 2] (col = tok//128)
            pos0a = sc.tile([128, BI], f32, tag="pos0a")
            pos1a = sc.tile([128, BI], f32, tag="pos1a")
            nc.gpsimd.dma_start(
                out=pos0a[:],
                in_=posd[0:1, 0:N].rearrange("a (p b) -> (a p) b", p=128))
            nc.gpsimd.dma_start(
                out=pos1a[:],
                in_=posd[0:1, N:2 * N].rearrange("a (p b) -> (a p) b", p=128))
            wall = sc.tile([128, BI], f32, tag="wall")
            k0 = sc.tile([128, BI], f32, tag="k0")
            nc.vector.tensor_scalar(out=k0[:], in0=pos0a[:], scalar1=float(CAP),
                                    scalar2=None, op0=AOP.is_lt)
            nc.vector.tensor_tensor(out=k0[:], in0=k0[:], in1=p0a[:], op=AOP.mult)
            k1 = sc.tile([128, BI], f32, tag="k1")
            nc.vector.tensor_scalar(out=k1[:], in0=pos1a[:], scalar1=float(CAP),
                                    scalar2=None, op0=AOP.is_lt)
            nc.vector.tensor_tensor(out=k1[:], in0=k1[:], in1=p1a[:], op=AOP.mult)
            nc.vector.tensor_tensor(out=wall[:], in0=k0[:], in1=k1[:], op=AOP.add)
            nc.gpsimd.dma_start(
                out=wd[:, :].rearrange("(p b) a -> p (b a)", p=128), in_=wall[:])
            wif = sc.tile([128, 2], f32, tag="wif")
            nc.vector.scalar_tensor_tensor(out=wif[:, 0:1], in0=cidb[:],
                                           scalar=float(TOKC), in1=iotap[:],
                                           op0=AOP.mult, op1=AOP.add)
            nc.vector.tensor_scalar_add(wif[:, 1:2], wif[:, 0:1], 128.0)
            nc.vector.tensor_copy(out=widx[:], in_=wif[:])
            for q in range(2):
                nc.gpsimd.indirect_dma_start(
                    out=wt[:, q:q + 1], out_offset=None,
                    in_=wd[:, :],
                    in_offset=bass.IndirectOffsetOnAxis(
                        ap=widx[:, q:q + 1], axis=0))
            wfa = sc.tile([128, 2], f32, tag="wfa")
            nc.vector.scalar_tensor_tensor(out=wfa[:, 0:1], in0=rkb[:],
                                           scalar=768.0, in1=iotap[:],
                                           op0=AOP.mult, op1=AOP.add)
            nc.vector.tensor_scalar_add(wfa[:, 1:2], wfa[:, 0:1], 128.0)
            nc.vector.tensor_copy(out=wiA[:], in_=wfa[:])
            wfb = sc.tile([128, 2], f32, tag="wfb")
            nc.vector.scalar_tensor_tensor(out=wfb[:, 0:1], in0=rkb[:],
                                           scalar=-256.0, in1=iotap[:],
                                           op0=AOP.mult, op1=AOP.add)
            nc.vector.tensor_scalar_add(wfb[:, 0:1], wfb[:, 0:1], 512.0)
            nc.vector.tensor_scalar_add(wfb[:, 1:2], wfb[:, 0:1], 128.0)
            nc.vector.tensor_copy(out=wiB[:], in_=wfb[:])

        # ============ Phase F2 + pair ReduceScatter per H-chunk ============
        # consume (epilogue of chunk ci-1) runs one chunk behind so the
        # gpsimd queue never blocks on an in-flight ReduceScatter
        with (tc.tile_pool(name="f2_w", bufs=12) as f2_w,
              tc.tile_pool(name="f2_ps", bufs=2, space="PSUM") as f2_ps,
              tc.tile_pool(name="pqs", bufs=2) as pqs,
              tc.tile_pool(name="cbp", bufs=2) as cbp):

            def consume(ci, off, w):
                # gather both pair partials for my 256 tokens out of the
                # AllGathered [2*TOKP, w] block pair, add, bias, scale
                for q in range(2):
                    ra = cbp.tile([128, 512], bf16, tag="ra")
                    rb2 = cbp.tile([128, 512], bf16, tag="rb2")
                    nc.gpsimd.indirect_dma_start(
                        out=ra[:, :w], out_offset=None,
                        in_=arq[ci][:, :],
                        in_offset=bass.IndirectOffsetOnAxis(
                            ap=wiA[:, q:q + 1], axis=0))
                    nc.gpsimd.indirect_dma_start(
                        out=rb2[:, :w], out_offset=None,
                        in_=arq[ci][:, :],
                        in_offset=bass.IndirectOffsetOnAxis(
                            ap=wiB[:, q:q + 1], axis=0))
                    hs = cbp.tile([128, 512], f32, tag="hs")
                    nc.vector.tensor_tensor(
                        out=hs[:, :w], in0=ra[:, :w], in1=rb2[:, :w],
                        op=AOP.add)
                    nc.vector.tensor_tensor(
                        out=hs[:, :w], in0=hs[:, :w],
                        in1=b2b[:, off:off + w], op=AOP.add)
                    o = cbp.tile([128, 512], f32, tag="o")
                    nc.vector.tensor_scalar_mul(
                        o[:, :w], hs[:, :w], wt[:, q:q + 1])
                    nc.gpsimd.dma_start(
                        out=out[q * 128:(q + 1) * 128, off:off + w],
                        in_=o[:, :w])

            for ci, (off, w) in enumerate(CHUNKS):
                psq = []
                for tt in range(4):
                    psq_t = f2_ps.tile([128, 512], f32, tag=f"f2_ps_{tt}")
                    psq.append(psq_t)
                for fc in range(FTH):
                    w2_t = f2_w.tile([128, 512], bf16, tag="w2_t")
                    eng = nc.sync if (fc % 2 == 0) else nc.scalar
                    eng.dma_start(out=w2_t[:, :w],
                                  in_=w2q[fc, :, off:off + w])
                    for tt in range(4):
                        nc.tensor.matmul(
                            out=psq[tt][:, :w],
                            lhsT=g[fc][:, tt * 128:(tt + 1) * 128],
                            rhs=w2_t[:, :w],
                            start=(fc == 0), stop=(fc == FTH - 1))
                for tt in range(4):
                    pb = pqs.tile([128, 512], bf16, tag=f"pb{tt % 2}")
                    nc.vector.tensor_copy(out=pb[:, :w], in_=psq[tt][:, :w])
                    nc.gpsimd.dma_start(
                        out=pq[ci][tt * 128:(tt + 1) * 128, :], in_=pb[:, :w])
                nc.gpsimd.collective_compute(
                    "AllGather", AOP.bypass, replica_groups=PRG,
                    ins=[pq[ci][:, :].opt()],
                    outs=[arq[ci][:, :].opt()])
                if ci > 0:
                    consume(ci - 1, *CHUNKS[ci - 1])
            consume(len(CHUNKS) - 1, *CHUNKS[-1])


# ======================== host-side glue ========================

_CACHE = {}


def _prep_inputs(hidden_states, router_w, w1, b1, w2, b2):
    x = np.asarray(hidden_states, np.float32).reshape(-1, H)
    xT = np.ascontiguousarray(x.T)
    w1Tm = np.asarray(w1, np.float32).T.astype(ml_dtypes.bfloat16)
    w2Tm = np.asarray(w2, np.float32).T.astype(ml_dtypes.bfloat16)
    w1tt = np.ascontiguousarray(
        w1Tm.reshape(16, 128, 64, 128).transpose(2, 1, 0, 3)).reshape(64, 128, 2048)
    w2qm = np.ascontiguousarray(w2Tm.reshape(64, 128, 2048))
    b1tf = np.ascontiguousarray(np.asarray(b1, np.float32).reshape(64, 128).T)
    base = {
        "rwT": np.ascontiguousarray(
            np.asarray(router_w, np.float32).T.reshape(16, 128, 8)
            .transpose(1, 0, 2).reshape(128, 128)),
        "b2r": np.asarray(b2, np.float32).reshape(1, H),
    }
    xtmf = np.ascontiguousarray(
        xT.reshape(16, 128, 16, 128).transpose(2, 1, 0, 3)).reshape(16, 128, 2048)
    xTb = xT.astype(ml_dtypes.bfloat16)
    ins = []
    for c in range(NCORES):
        hh = c % 2          # ffn half
        p = c // 2          # pair (owns tokens [512p, 512p+512))
        m = dict(base)
        m["w1tt"] = np.ascontiguousarray(w1tt[hh * FTH:(hh + 1) * FTH])
        m["w2q"] = np.ascontiguousarray(w2qm[hh * FTH:(hh + 1) * FTH])
        m["b1t"] = np.ascontiguousarray(b1tf[:, hh * FTH:(hh + 1) * FTH])
        m["xtm"] = np.ascontiguousarray(xtmf[2 * c:2 * c + 2])
        # xtt[pp, ht*512 + t] = x[512p + t, ht*128 + pp]
        m["xtt"] = np.ascontiguousarray(
            xTb[:, p * TOKP:(p + 1) * TOKP].reshape(HT, 128, TOKP)
            .transpose(1, 0, 2).reshape(128, HT * TOKP))
        m["cid"] = np.full((1, 1), float(c), np.float32)
        m["rk"] = np.full((1, 1), float(c % 2), np.float32)
        ins.append(m)
    return ins


def _get_nc():
    if "nc" not in _CACHE:
        apply()  # tile drain patch
        nc = bass.Bass(num_devices=NCORES)
        build_moe(nc)
        split_multi_waits(nc)
        _CACHE["nc"] = nc
    return _CACHE["nc"]


def kernel(hidden_states, router_w, w1, b1, w2, b2):
    from concourse.bass_utils import run_bass_kernel_spmd

    orig_shape = np.asarray(hidden_states).shape
    nc = _get_nc()
    ins = _prep_inputs(hidden_states, router_w, w1, b1, w2, b2)
    res = run_bass_kernel_spmd(nc, ins, core_ids=list(range(NCORES)))
    full = np.concatenate([res.results[c]["out"] for c in range(NCORES)], axis=0)
    return full.reshape(orig_shape).astype(np.float32)


# revision 29
# speedup vs baseline: 1.2963x; 1.1039x over previous
"""Trainium2 8-core MoE layer kernel (token x ffn sharded dense FFN, Bass/Tile).

Contract: kernel(**inputs) takes the full unsharded numpy inputs of the
MoE reference (hidden_states, router_w, w1, b1, w2, b2) and returns the
full [2, 1024, 2048] float32 output.

Key identity: the reference's experts all share one FFN (w1/b1/w2/b2 are
not per-expert), so for every kept (token, k) slot the expert output is
FFN(x[t]) and the combine collapses to
    out[t] = (sum_k kept_k(t) * gate_k(t)) * FFN(x[t]).
Routing therefore only determines a per-token scalar; the FFN itself is
a dense [N, H] pass.

Sharding: cores form pairs (2p, 2p+1) owning tokens [512p, 512p+512).
Within a pair the ffn dim F is split in half (4096 per core), so each
core streams only 32 MB of weights (the kernel is HBM-bound at ~220 GB/s
per core, while the PE floor is ~134 us per GEMM phase). Each core
computes partial y = gelu(x W1h^T + b1h) W2h^T for all 512 pair tokens;
a per-H-chunk bf16 ReduceScatter(add) over the pair then hands every
core the finished rows of its own 256 tokens (= global tokens
[256c, 256c+256), matching the host-side concat).

The fp32 router runs on each core's own 256 tokens; an AllGather shares
the [N, 4] routing decisions; a replicated capacity scan produces the
per-token weight w(t), indirect-gathered per core. That whole chain
rides on vector/gpsimd during F1; its two tiny matmul groups issue
between F1 and F2 on the tensor queue.
"""
import numpy as np
import ml_dtypes

import concourse.bass as bass
import concourse.mybir as mybir
import concourse.tile as tile

_PATCH_DOC = """Patch TileContext._drain_and_barrier: the stock version stuffs every
outstanding semaphore wait onto one SP Drain instruction; the installed
walrus rejects >1 sync wait per non-EventSemaphore instruction
("Too many sync wait commands"). Split the waits across a chain of SP
nops, then drain/barrier as before."""
import concourse.tile as tile_mod
from concourse.vector_clock import ScopedClock


def _patched_drain_and_barrier(self, tick_clock, wait_clock):
    nc = self.nc
    carrier = nc.sync.nop(nofuse=True, hint="drain_wait_carrier")
    wait_clock.add_sem_waits(
        carrier.ins, ScopedClock({None: tick_clock.global_clock})
    )
    waits = list(carrier.ins.sync_info.on_wait)
    if len(waits) > 1:
        carrier.ins.sync_info.on_wait = waits[:1]
        import bass_rust as _br
        for w in waits[1:]:
            extra = nc.sync.nop(nofuse=True, hint="drain_wait_carrier")
            extra.ins.sync_info = _br.SyncInfo(on_wait=[w], on_update=[])

    nc.sync.drain()
    nc.all_engine_barrier()
    assert self.sems is not None
    popped = nc._tile_sem_poison_stack.pop()
    assert popped is self._sem_poison
    nc.clear_and_free_semaphores(list(self.sems.allocated().values()))
    nc.all_engine_barrier()


def apply():
    tile_mod.TileContext._drain_and_barrier = _patched_drain_and_barrier


import concourse.mybir as mybir
import bass_rust as _br


def split_multi_waits(nc):
    """Walrus in this container accepts at most ONE sync wait per
    instruction. Hoist extra waits onto same-engine NoOps inserted
    immediately before the offending instruction."""
    ctr = 0
    for f in nc.m.functions:
        for b in f.blocks:
            insts = b.instructions
            need = any(
                inst.sync_info is not None and len(inst.sync_info.on_wait) > 1
                for inst in insts
            )
            if not need:
                continue
            out = []
            for inst in insts:
                si = inst.sync_info
                if si is not None and len(si.on_wait) > 1:
                    waits = list(si.on_wait)
                    for w in waits[:-1]:
                        nop = mybir.InstNoOp(name=f"I-wsplit-{ctr}", ins=[], outs=[])
                        ctr += 1
                        nop.engine = inst.engine
                        nop.sync_info = _br.SyncInfo(on_wait=[w], on_update=[])
                        out.append(nop)
                    si.on_wait = waits[-1:]
                out.append(inst)
            b.instructions = out
    return ctr


E, TOPK, CAP, H, F, N, NCORES = 8, 2, 512, 2048, 8192, 2048, 8
HT = H // 128                 # 16 hidden tiles
FH = F // 2                   # 4096 ffn columns per core
FTH = FH // 128               # 32 local ffn tiles
TOKC = N // NCORES            # 256 output tokens per core
TOKP = 2 * TOKC               # 512 tokens per pair
BI = N // 128                 # 16
NSEG = 4
SEGL = 2 * N // NSEG
# F2 H-chunks (PSUM: 4 token tiles x 512 fp32 x double-buffer = 8 banks)
CHUNKS = [(0, 512), (512, 512), (1024, 512), (1536, 512)]

f32 = mybir.dt.float32
f16 = mybir.dt.float16
bf16 = mybir.dt.bfloat16
i32 = mybir.dt.int32
AOP = mybir.AluOpType
AFT = mybir.ActivationFunctionType
AX = mybir.AxisListType


def build_moe(nc: bass.Bass):
    xtm = nc.dram_tensor("xtm", [2, 128, H], f32, kind="ExternalInput")
    rk = nc.dram_tensor("rk", [1, 1], f32, kind="ExternalInput")
    xtt = nc.dram_tensor("xtt", [128, HT * TOKP], bf16, kind="ExternalInput")
    rwT = nc.dram_tensor("rwT", [128, HT * E], f32, kind="ExternalInput")
    w1T = nc.dram_tensor("w1tt", [FTH, 128, HT * 128], bf16, kind="ExternalInput")
    w2q = nc.dram_tensor("w2q", [FTH, 128, H], bf16, kind="ExternalInput")
    b1t = nc.dram_tensor("b1t", [128, FTH], f32, kind="ExternalInput")
    b2r = nc.dram_tensor("b2r", [1, H], f32, kind="ExternalInput")
    cid = nc.dram_tensor("cid", [1, 1], f32, kind="ExternalInput")
    out = nc.dram_tensor("out", [TOKP, H], f32, kind="ExternalOutput")

    rloc = nc.dram_tensor("rloc", [TOKC, 4], f32)
    rall = nc.dram_tensor("rall", [N, 4], f32, addr_space="Shared")
    ebuf8 = nc.dram_tensor("ebuf8", [E, 2 * N], f32)
    e32d = nc.dram_tensor("e32d", [1, 32], f32)
    posd = nc.dram_tensor("posd", [1, 2 * N], f32)
    wd = nc.dram_tensor("wd", [N, 1], f32)

    with tile.TileContext(nc, num_cores=NCORES) as tc:
        with tc.tile_pool(name="persist", bufs=1) as persist:
            _body(nc, tc, persist, xtm, rk, xtt, rwT, w1T, w2q, b1t, b2r,
                  cid, out, rloc, rall, ebuf8, e32d, posd, wd)
    return nc


def _body(nc, tc, persist, xtm, rk, xtt, rwT, w1T, w2q, b1t, b2r, cid, out,
          rloc, rall, ebuf8, e32d, posd, wd):
    RG = [list(range(NCORES))]
    PRG = [[2 * p, 2 * p + 1] for p in range(NCORES // 2)]
    sc = persist

    # ---- persistent tiles ----
    b2b = persist.tile([128, H], f32, tag="b2b")
    cidb = persist.tile([128, 1], f32, tag="cidb")
    b1sb = persist.tile([128, FTH], f32, tag="b1sb")
    rws = persist.tile([128, HT * E], f32, tag="rws")
    xcT = persist.tile([128, HT * TOKP], bf16, tag="xcT")
    iotap = persist.tile([128, 1], f32, tag="iotap")
    rkb = persist.tile([128, 1], f32, tag="rkb")
    # w(t) for all 4 pair token tiles (col q = pair token q*128 + p)
    wt = persist.tile([128, 4], f32, tag="wt")
    widx = persist.tile([128, 4], i32, tag="widx")
    # b2 contribution only from rank 0 (host sums the two pair partials)
    b2s = persist.tile([128, H], f32, tag="b2s")

    # router x first, then F1 x, split across the two fast queues so the
    # router and F1 start as early as possible; everything else on gpsimd
    QX = HT * TOKP // 2
    nc.gpsimd.dma_start(out=rws[:], in_=rwT[:, :])
    nc.gpsimd.dma_start(out=cidb[:], in_=cid[0:1, :].partition_broadcast(128).opt())
    nc.gpsimd.dma_start(out=rkb[:], in_=rk[0:1, :].partition_broadcast(128).opt())
    nc.gpsimd.dma_start(out=b1sb[:], in_=b1t[:, :])
    nc.gpsimd.dma_start(out=b2b[:], in_=b2r[0:1, :].partition_broadcast(128).opt())
    ip = persist.tile([128, 1], i32, tag="ip")
    nc.gpsimd.iota(ip[:], pattern=[[0, 1]], base=0, channel_multiplier=1)
    nc.vector.tensor_copy(out=iotap[:], in_=ip[:])
    mk1 = persist.tile([128, 1], f32, tag="mk1")
    nc.vector.tensor_scalar_mul(mk1[:], rkb[:], -1.0)
    nc.vector.tensor_scalar_add(mk1[:], mk1[:], 1.0)
    nc.vector.tensor_scalar_mul(b2s[:], b2b[:], mk1[:])

    # ============ Phase R: sharded router (own 256 tokens, fp32) ============
    with (tc.tile_pool(name="r_x", bufs=2) as r_x,
          tc.tile_pool(name="r_ps", bufs=2, space="PSUM") as r_ps,
          tc.tile_pool(name="r_sb", bufs=2) as r_sb):
        for tt2 in range(2):
            xt_t = r_x.tile([128, H], f32, tag="xt_t")
            for qq in range(2):
                eng = nc.sync if qq == 0 else nc.scalar
                eng.dma_start(
                    out=xt_t[:, qq * (H // 2):(qq + 1) * (H // 2)],
                    in_=xtm[tt2, :, qq * (H // 2):(qq + 1) * (H // 2)])
            ps = r_ps.tile([128, E], f32, tag="r_ps")
            for hc in range(HT):
                nc.tensor.matmul(
                    out=ps[:], lhsT=xt_t[:, hc * 128:(hc + 1) * 128],
                    rhs=rws[:, hc * E:(hc + 1) * E],
                    start=(hc == 0), stop=(hc == HT - 1))
            lsb = r_sb.tile([128, E], f32, tag="lsb")
            nc.vector.tensor_copy(out=lsb[:], in_=ps[:])
            mx = r_sb.tile([128, 1], f32, tag="mx")
            nc.vector.tensor_reduce(out=mx[:], in_=lsb[:], op=AOP.max, axis=AX.X)
            nm = r_sb.tile([128, 1], f32, tag="nm")
            nc.vector.tensor_scalar_mul(nm[:], mx[:], -1.0)
            ex = r_sb.tile([128, E], f32, tag="ex")
            ssum = r_sb.tile([128, 1], f32, tag="ssum")
            nc.scalar.activation(out=ex[:], in_=lsb[:], func=AFT.Exp,
                                 bias=nm[:], scale=1.0, accum_out=ssum[:])
            rcp = r_sb.tile([128, 1], f32, tag="rcp")
            nc.vector.reciprocal(out=rcp[:], in_=ssum[:])
            pr = r_sb.tile([128, E], f32, tag="pr")
            nc.vector.tensor_scalar_mul(pr[:], ex[:], rcp[:])
            mx8 = r_sb.tile([128, 8], f32, tag="mx8")
            ix8 = r_sb.tile([128, 8], mybir.dt.uint32, tag="ix8")
            nc.vector.max_with_indices(out_max=mx8[:], out_indices=ix8[:],
                                       in_=pr[:])
            rv = r_sb.tile([128, 4], f32, tag="rv")
            nc.vector.tensor_copy(out=rv[:, 0:1], in_=ix8[:, 0:1])
            nc.vector.tensor_copy(out=rv[:, 1:2], in_=ix8[:, 1:2])
            nc.vector.tensor_copy(out=rv[:, 2:3], in_=mx8[:, 0:1])
            nc.vector.tensor_copy(out=rv[:, 3:4], in_=mx8[:, 1:2])
            nc.gpsimd.dma_start(out=rloc[tt2 * 128:(tt2 + 1) * 128, :], in_=rv[:])
        nc.sync.dma_start(out=xcT[:, 0:QX], in_=xtt[:, 0:QX])
        nc.scalar.dma_start(out=xcT[:, QX:2 * QX], in_=xtt[:, QX:2 * QX])
        nc.gpsimd.collective_compute(
            "AllGather", AOP.bypass,
            replica_groups=RG,
            ins=[rloc[:, :].opt()],
            outs=[rall[:, :].opt()])

    # contiguous per-partition load of the gathered routing, (p, b) layout
    rb = persist.tile([128, 16 * 4], f32, tag="rb")
    nc.gpsimd.dma_start(
        out=rb[:], in_=rall[:, :].rearrange("(p b) c -> p (b c)", p=128))
    rbv = rb[:].rearrange("p (b c) -> p b c", c=4)
    e0a = persist.tile([128, BI], f32, tag="e0a")
    e1a = persist.tile([128, BI], f32, tag="e1a")
    p0a = persist.tile([128, BI], f32, tag="p0a")
    p1a = persist.tile([128, BI], f32, tag="p1a")
    nc.vector.tensor_copy(out=e0a[:], in_=rbv[:, :, 0].opt())
    nc.vector.tensor_copy(out=e1a[:], in_=rbv[:, :, 1].opt())
    nc.vector.tensor_copy(out=p0a[:], in_=rbv[:, :, 2].opt())
    nc.vector.tensor_copy(out=p1a[:], in_=rbv[:, :, 3].opt())

    # ============ Phase S: one-hot + 4-way segmented scan (fp16) ============
    # pack expert ids, roundtrip through DRAM to get the (k, t)-ordered
    # row (t = p*16 + b), then a broadcast load into [32, 1024]: partition
    # (e, seg) scans its 1024-long segment; segment offsets fixed up via a
    # small triangular matmul over the per-segment totals (issued after F1
    # on the tensor queue; all deps are ready long before it reaches PE).
    ip32 = sc.tile([32, 1], i32, tag="ip32")
    ip32f = sc.tile([32, 1], f32, tag="ip32f")
    nc.gpsimd.iota(ip32[:], pattern=[[0, 1]], base=0, channel_multiplier=1)
    nc.vector.tensor_copy(out=ip32f[:], in_=ip32[:])
    eri = sc.tile([1, 32], i32, tag="eri")
    nc.gpsimd.iota(eri[:], pattern=[[1, E], [0, NSEG]], base=0,
                   channel_multiplier=0)
    erf = sc.tile([1, 32], f32, tag="erf")
    nc.vector.tensor_copy(out=erf[:], in_=eri[:])
    nc.gpsimd.dma_start(out=e32d[0:1, :], in_=erf[:])
    eidx = sc.tile([32, 1], f32, tag="eidx")
    nc.gpsimd.dma_start(
        out=eidx[:, :],
        in_=e32d[0:1, :].rearrange("a (c u) -> (a c) u", u=1))
    sidx = sc.tile([32, 1], f32, tag="sidx")
    nc.vector.scalar_tensor_tensor(out=sidx[:], in0=eidx[:],
                                   scalar=-float(NSEG), in1=ip32f[:],
                                   op0=AOP.mult, op1=AOP.add)
    # Mt[p', p] = same expert and seg(p') < seg(p): exclusive prefix mask
    jmi = sc.tile([32, 32], i32, tag="jmi")
    nc.gpsimd.iota(jmi[:], pattern=[[0, E], [1, NSEG]], base=0,
                   channel_multiplier=0)
    jm = sc.tile([32, 32], f32, tag="jm")
    nc.vector.tensor_copy(out=jm[:], in_=jmi[:])
    eci = sc.tile([32, 32], i32, tag="eci")
    nc.gpsimd.iota(eci[:], pattern=[[1, E], [0, NSEG]], base=0,
                   channel_multiplier=0)
    ec = sc.tile([32, 32], f32, tag="ec")
    nc.vector.tensor_copy(out=ec[:], in_=eci[:])
    Mt = sc.tile([32, 32], f16, tag="Mt")
    me32 = sc.tile([32, 32], f32, tag="me32")
    nc.vector.tensor_scalar(out=me32[:], in0=ec[:],
                            scalar1=eidx[:], scalar2=None, op0=AOP.is_equal)
    ms32 = sc.tile([32, 32], f32, tag="ms32")
    nc.vector.tensor_scalar(out=ms32[:], in0=jm[:],
                            scalar1=sidx[:], scalar2=None, op0=AOP.is_gt)
    nc.vector.tensor_tensor(out=Mt[:], in0=me32[:], in1=ms32[:],
                            op=AOP.mult)
    sel4 = sc.tile([32, NSEG], f16, tag="sel4")
    iseg = sc.tile([32, NSEG], i32, tag="iseg")
    nc.gpsimd.iota(iseg[:], pattern=[[1, NSEG]], base=0, channel_multiplier=0)
    isegf = sc.tile([32, NSEG], f32, tag="isegf")
    nc.vector.tensor_copy(out=isegf[:], in_=iseg[:])
    nc.vector.tensor_scalar(out=sel4[:], in0=isegf[:], scalar1=sidx[:],
                            scalar2=None, op0=AOP.is_equal)

    e01 = sc.tile([128, 32], f32, tag="e01")
    nc.vector.tensor_copy(out=e01[:, 0:16], in_=e0a[:])
    nc.vector.tensor_copy(out=e01[:, 16:32], in_=e1a[:])
    for e in range(E):
        nc.gpsimd.dma_start(
            out=ebuf8[e:e + 1, :].rearrange(
                "a (k p b) -> (a p) k b", k=2, p=128),
            in_=e01[:].rearrange("p (k b) -> p k b", k=2))
    ohsrc = sc.tile([32, SEGL], f32, tag="ohsrc")
    nc.gpsimd.dma_start(
        out=ohsrc[:],
        in_=ebuf8[:, :].rearrange("e (s c) -> (e s) c", s=NSEG))
    ohcat = sc.tile([32, SEGL], f16, tag="ohcat")
    nc.vector.tensor_scalar(out=ohcat[:], in0=ohsrc[:], scalar1=eidx[:],
                            scalar2=None, op0=AOP.is_equal)
    ones2n = sc.tile([32, SEGL], f16, tag="ones2n")
    nc.vector.memset(ones2n[:], 1.0)
    cum = sc.tile([32, SEGL], f16, tag="cum")
    nc.vector.tensor_tensor_scan(out=cum[:], data0=ones2n[:], data1=ohcat[:],
                                 initial=0.0, op0=AOP.mult, op1=AOP.add)
    tot32 = sc.tile([32, 1], f16, tag="tot32")
    with nc.allow_low_precision(reason="segment counts <= 1024, f16-exact"):
        nc.vector.tensor_reduce(out=tot32[:], in_=ohcat[:], op=AOP.add,
                                axis=AX.X)

    # ============ Phase F1 (dense, 512 pair tokens, local F half) ============
    with tc.tile_pool(name="g", bufs=1) as g_pool:
        g = []
        with (tc.tile_pool(name="f1_w", bufs=4) as f1_w,
              tc.tile_pool(name="f1_ps", bufs=2, space="PSUM") as f1_ps):
            for ft in range(FTH):
                w1_t = f1_w.tile([128, HT * 128], bf16, tag="w1_t")
                QW = HT * 128 // 2
                for qq in range(2):
                    eng = nc.sync if qq == 0 else nc.scalar
                    eng.dma_start(
                        out=w1_t[:, qq * QW:(qq + 1) * QW],
                        in_=w1T[ft, :, qq * QW:(qq + 1) * QW])
                ps = f1_ps.tile([128, TOKP], f32, tag="f1_ps")
                for hc in range(HT):
                    nc.tensor.matmul(
                        out=ps[:], lhsT=w1_t[:, hc * 128:(hc + 1) * 128],
                        rhs=xcT[:, hc * TOKP:(hc + 1) * TOKP],
                        start=(hc == 0), stop=(hc == HT - 1))
                gt = g_pool.tile([128, TOKP], bf16, tag=f"g_{ft}")
                nc.scalar.activation(out=gt[:], in_=ps[:], func=AFT.Gelu,
                                     bias=b1sb[:, ft:ft + 1], scale=1.0)
                g.append(gt)

        # ---- scan fixup (tensor) + per-token weight chain ----
        with tc.tile_pool(name="s_ps", bufs=2, space="PSUM") as s_ps:
            offp = s_ps.tile([32, 1], f32, tag="offp")
            nc.tensor.matmul(out=offp[:], lhsT=Mt[:], rhs=tot32[:],
                             start=True, stop=True)
            off32 = sc.tile([32, 1], f32, tag="off32")
            nc.vector.tensor_copy(out=off32[:], in_=offp[:])
            cumf = sc.tile([32, SEGL], f16, tag="cumf")
            nc.vector.tensor_scalar(out=cumf[:], in0=cum[:], scalar1=off32[:],
                                    scalar2=None, op0=AOP.add)
            ohcum = sc.tile([32, SEGL], f16, tag="ohcum")
            nc.vector.tensor_tensor(out=ohcum[:], in0=ohcat[:], in1=cumf[:],
                                    op=AOP.mult)
            posrow = sc.tile([1, 2 * N], f32, tag="posrow")
            for s in range(NSEG):
                for ch in range(SEGL // 512):
                    pps = s_ps.tile([1, 512], f32, tag="pps")
                    nc.tensor.matmul(out=pps[:], lhsT=sel4[:, s:s + 1],
                                     rhs=ohcum[:, ch * 512:(ch + 1) * 512],
                                     start=True, stop=True)
                    nc.vector.tensor_scalar_add(
                        posrow[:, s * SEGL + ch * 512:s * SEGL + (ch + 1) * 512],
                        pps[:], -1.0)
            nc.gpsimd.dma_start(out=posd[:, 0:N], in_=posrow[:, 0:N])
            nc.gpsimd.dma_start(out=posd[:, N:2 * N], in_=posrow[:, N:2 * N])

            # w(t) = p0*(pos0<CAP) + p1*(pos1<CAP) for all tokens -> wd,
            # then indirect-gather own 256 into wt[128, 2] (col = tok//128)
            pos0a = sc.tile([128, BI], f32, tag="pos0a")
            pos1a = sc.tile([128, BI], f32, tag="pos1a")
            nc.gpsimd.dma_start(
                out=pos0a[:],
                in_=posd[0:1, 0:N].rearrange("a (p b) -> (a p) b", p=128))
            nc.gpsimd.dma_start(
                out=pos1a[:],
                in_=posd[0:1, N:2 * N].rearrange("a (p b) -> (a p) b", p=128))
            wall = sc.tile([128, BI], f32, tag="wall")
            k0 = sc.tile([128, BI], f32, tag="k0")
            nc.vector.tensor_scalar(out=k0[:], in0=pos0a[:], scalar1=float(CAP),
                                    scalar2=None, op0=AOP.is_lt)
            nc.vector.tensor_tensor(out=k0[:], in0=k0[:], in1=p0a[:], op=AOP.mult)
            k1 = sc.tile([128, BI], f32, tag="k1")
            nc.vector.tensor_scalar(out=k1[:], in0=pos1a[:], scalar1=float(CAP),
                                    scalar2=None, op0=AOP.is_lt)
            nc.vector.tensor_tensor(out=k1[:], in0=k1[:], in1=p1a[:], op=AOP.mult)
            nc.vector.tensor_tensor(out=wall[:], in0=k0[:], in1=k1[:], op=AOP.add)
            nc.gpsimd.dma_start(
                out=wd[:, :].rearrange("(p b) a -> p (b a)", p=128), in_=wall[:])
            # widx[p, q] = 512*pair + q*128 + p = 256*(cid - rk) + q*128 + p
            wif = sc.tile([128, 4], f32, tag="wif")
            nc.vector.scalar_tensor_tensor(out=wif[:, 0:1], in0=cidb[:],
                                           scalar=float(TOKC), in1=iotap[:],
                                           op0=AOP.mult, op1=AOP.add)
            nc.vector.scalar_tensor_tensor(out=wif[:, 0:1], in0=rkb[:],
                                           scalar=-float(TOKC), in1=wif[:, 0:1],
                                           op0=AOP.mult, op1=AOP.add)
            for q in range(1, 4):
                nc.vector.tensor_scalar_add(wif[:, q:q + 1], wif[:, q - 1:q],
                                            128.0)
            nc.vector.tensor_copy(out=widx[:], in_=wif[:])
            for q in range(4):
                nc.gpsimd.indirect_dma_start(
                    out=wt[:, q:q + 1], out_offset=None,
                    in_=wd[:, :],
                    in_offset=bass.IndirectOffsetOnAxis(
                        ap=widx[:, q:q + 1], axis=0))

        # ============ Phase F2 + scaled-partial epilogue, H-chunk-major ====
        # each core emits w(t) * (g_half W2h^T + [rank==0]*b2) for all 512
        # pair tokens; the host sums the two fp32 partials of each pair
        with (tc.tile_pool(name="f2_w", bufs=12) as f2_w,
              tc.tile_pool(name="f2_ps", bufs=2, space="PSUM") as f2_ps,
              tc.tile_pool(name="cbp", bufs=4) as cbp):
            for ci, (off, w) in enumerate(CHUNKS):
                psq = []
                for tt in range(4):
                    psq_t = f2_ps.tile([128, 512], f32, tag=f"f2_ps_{tt}")
                    psq.append(psq_t)
                for fc in range(FTH):
                    w2_t = f2_w.tile([128, 512], bf16, tag="w2_t")
                    eng = nc.sync if (fc % 2 == 0) else nc.scalar
                    eng.dma_start(out=w2_t[:, :w],
                                  in_=w2q[fc, :, off:off + w])
                    for tt in range(4):
                        nc.tensor.matmul(
                            out=psq[tt][:, :w],
                            lhsT=g[fc][:, tt * 128:(tt + 1) * 128],
                            rhs=w2_t[:, :w],
                            start=(fc == 0), stop=(fc == FTH - 1))
                for tt in range(4):
                    hs = cbp.tile([128, 512], f32, tag="hs")
                    nc.vector.tensor_tensor(
                        out=hs[:, :w], in0=psq[tt][:, :w],
                        in1=b2s[:, off:off + w], op=AOP.add)
                    o = cbp.tile([128, 512], f32, tag="o")
                    nc.vector.tensor_scalar_mul(
                        o[:, :w], hs[:, :w], wt[:, tt:tt + 1])
                    eng = nc.gpsimd if tt % 2 == 0 else nc.scalar
                    eng.dma_start(
                        out=out[tt * 128:(tt + 1) * 128, off:off + w],
                        in_=o[:, :w])


# ======================== host-side glue ========================

_CACHE = {}


def _prep_inputs(hidden_states, router_w, w1, b1, w2, b2):
    x = np.asarray(hidden_states, np.float32).reshape(-1, H)
    xT = np.ascontiguousarray(x.T)
    w1Tm = np.asarray(w1, np.float32).T.astype(ml_dtypes.bfloat16)
    w2Tm = np.asarray(w2, np.float32).T.astype(ml_dtypes.bfloat16)
    w1tt = np.ascontiguousarray(
        w1Tm.reshape(16, 128, 64, 128).transpose(2, 1, 0, 3)).reshape(64, 128, 2048)
    w2qm = np.ascontiguousarray(w2Tm.reshape(64, 128, 2048))
    b1tf = np.ascontiguousarray(np.asarray(b1, np.float32).reshape(64, 128).T)
    base = {
        "rwT": np.ascontiguousarray(
            np.asarray(router_w, np.float32).T.reshape(16, 128, 8)
            .transpose(1, 0, 2).reshape(128, 128)),
        "b2r": np.asarray(b2, np.float32).reshape(1, H),
    }
    xtmf = np.ascontiguousarray(
        xT.reshape(16, 128, 16, 128).transpose(2, 1, 0, 3)).reshape(16, 128, 2048)
    xTb = xT.astype(ml_dtypes.bfloat16)
    ins = []
    for c in range(NCORES):
        hh = c % 2          # ffn half
        p = c // 2          # pair (owns tokens [512p, 512p+512))
        m = dict(base)
        m["w1tt"] = np.ascontiguousarray(w1tt[hh * FTH:(hh + 1) * FTH])
        m["w2q"] = np.ascontiguousarray(w2qm[hh * FTH:(hh + 1) * FTH])
        m["b1t"] = np.ascontiguousarray(b1tf[:, hh * FTH:(hh + 1) * FTH])
        m["xtm"] = np.ascontiguousarray(xtmf[2 * c:2 * c + 2])
        # xtt[pp, ht*512 + t] = x[512p + t, ht*128 + pp]
        m["xtt"] = np.ascontiguousarray(
            xTb[:, p * TOKP:(p + 1) * TOKP].reshape(HT, 128, TOKP)
            .transpose(1, 0, 2).reshape(128, HT * TOKP))
        m["cid"] = np.full((1, 1), float(c), np.float32)
        m["rk"] = np.full((1, 1), float(c % 2), np.float32)
        ins.append(m)
    return ins


def _get_nc():
    if "nc" not in _CACHE:
        apply()  # tile drain patch
        nc = bass.Bass(num_devices=NCORES)
        build_moe(nc)
        split_multi_waits(nc)
        _CACHE["nc"] = nc
    return _CACHE["nc"]


def kernel(hidden_states, router_w, w1, b1, w2, b2):
    from concourse.bass_utils import run_bass_kernel_spmd

    orig_shape = np.asarray(hidden_states).shape
    nc = _get_nc()
    ins = _prep_inputs(hidden_states, router_w, w1, b1, w2, b2)
    res = run_bass_kernel_spmd(nc, ins, core_ids=list(range(NCORES)))
    # each pair's two cores return fp32 partials over their ffn halves
    full = np.concatenate(
        [res.results[2 * p]["out"] + res.results[2 * p + 1]["out"]
         for p in range(NCORES // 2)], axis=0)
    return full.reshape(orig_shape).astype(np.float32)
